# revision 45
# baseline (speedup 1.0000x reference)
"""Trainium2 Bass kernel for nn_Block_69191923139027 (dense_transformer).

Sharding: 8 cores; core k owns Feebler/Booster rows i in [8k, 8k+8), which
is exactly tokens [256k, 256k+256) per batch, so the Booster needs only
locally-computed h (no h AllGather). Two tiny AllReduces stitch the global
k/v sums and softmax denominators.

v5: quarter-pipelined x stream keeps PE warm (2.4GHz), constant [128,2]
feebler stationary, AllReduce of pre-projected k/v partials, preloaded
activation tables, 16-bit stationaries/moving operands everywhere,
softmax 1/z applied to e (bounded <=1) instead of the proj weights,
booster multiplies read PSUM directly split across DVE and GpSimd.

Self-contained: hardcodes all shapes; no sibling imports.
"""

import numpy as np

import concourse.bacc as bacc
import concourse.mybir as mybir
import concourse.tile as tile
from concourse.bass_utils import run_bass_kernel_spmd

N_CORES = 8
B, T, SD, NE = 4, 2048, 64, 4096
H, HS, FH = 8, 8, 256
EPS = 1e-5
IPC = SD // N_CORES          # 8 feebler rows per core
TLOC = B * IPC * 32          # 1024 local tokens; hT col = b*256 + a*8 + i
DT = mybir.dt.float32
F16 = mybir.dt.float16
RG = [list(range(N_CORES))]
ESHIFT = 64.0                # softmax logit shift (max |logit| ~ 71)

_CACHE = {}
CHH = [(0, 512), (512, 512), (1024, 512), (1536, 256), (1792, 256)]


def _build_nc():
    nc = bacc.Bacc("TRN2", target_bir_lowering=False, debug=False,
                   num_devices=N_CORES)
    A = mybir.AluOpType
    AF = mybir.ActivationFunctionType

    tn = {}
    # x/fw/bw are host-pretransposed to partition-major so every stream
    # DMA is a 2D contiguous copy (128 descriptors, not 2048).
    tn["x"] = nc.dram_tensor("x", [128, B * 4 * T], F16,
                             kind="ExternalInput")
    tn["fw"] = nc.dram_tensor("fw", [128, 4 * T], F16, kind="ExternalInput")
    tn["bw"] = nc.dram_tensor("bw", [128, 4 * T], F16, kind="ExternalInput")
    # all small weights packed into two tensors (2 DMA issues, not 13)
    tn["pk32"] = nc.dram_tensor("pk32", [128, 262], DT, kind="ExternalInput")
    tn["pk16"] = nc.dram_tensor("pk16", [128, 3200], F16,
                                kind="ExternalInput")
    out = nc.dram_tensor("out", [B * IPC * SD, T], F16, kind="ExternalOutput")
    if _CACHE.get("debug"):
        tn["dbg_h"] = nc.dram_tensor("dbg_h", [32, T], DT,
                                     kind="ExternalOutput")
        tn["dbg_hT"] = nc.dram_tensor("dbg_hT", [64, TLOC], F16,
                                      kind="ExternalOutput")
        tn["dbg_y1"] = nc.dram_tensor("dbg_y1", [64, TLOC], F16,
                                      kind="ExternalOutput")
        tn["dbg_e2"] = nc.dram_tensor("dbg_e2", [64, TLOC], F16,
                                      kind="ExternalOutput")
        tn["dbg_kv"] = nc.dram_tensor("dbg_kv", [128, 3 * B], DT,
                                      kind="ExternalOutput")
        tn["dbg_h3"] = nc.dram_tensor("dbg_h3", [64, TLOC], F16,
                                      kind="ExternalOutput")
        tn["dbg_h2h"] = nc.dram_tensor("dbg_h2h", [32, T], F16,
                                       kind="ExternalOutput")

    with tile.TileContext(nc) as tc:
        _body(nc, tc, tn, out, A, AF)
    nc.compile()
    return nc


def _body(nc, tc, tn, out, A, AF):
    X = mybir.AxisListType.X

    with tc.tile_pool(name="wconst", bufs=1) as wp, \
         tc.tile_pool(name="mid", bufs=1) as mp, \
         tc.tile_pool(name="bwpool", bufs=1) as bwp, \
         tc.tile_pool(name="dram", bufs=1, space="DRAM") as dp:

        # ---- on-chip constants ----
        ones64 = wp.tile([SD, 2], F16, tag="ones64")
        nc.vector.memset(ones64[:], 1.0 / SD)
        ones1r = wp.tile([1, SD], F16, tag="ones1r")
        nc.vector.memset(ones1r[:], 1.0)
        epsv = wp.tile([1, 1], DT, tag="epsv")
        nc.vector.memset(epsv[:], EPS)

        # preload activation tables (Exp/Sqrt/Square/Relu/Identity) so no
        # ACT_TABLE_LOAD lands on the critical path later.
        warm = wp.tile([1, 2], DT, tag="warm")
        nc.vector.memset(warm[:], 1.0)
        for fn in (AF.Identity, AF.Square, AF.Sqrt, AF.Relu, AF.Exp):
            nc.scalar.activation(warm[:], warm[:], fn)

        # warm-up collective: absorbs the one-time CC setup + barrier
        # while the feebler streams.
        ccw_i = dp.tile([1, 1], DT, tag="ccw_i")
        ccw_o = dp.tile([N_CORES, 1], DT, tag="ccw_o", addr_space="Shared")
        warm1 = wp.tile([1, 1], DT, tag="warm1")
        nc.vector.memset(warm1[:], 0.0)
        nc.sync.dma_start(ccw_i[:], warm1[:])
        nc.gpsimd.collective_compute("AllGather", A.bypass,
                                     ins=[ccw_i[:]], outs=[ccw_o[:]],
                                     replica_groups=RG)

        # ---- small weights, two packed tiles (2 DMA issues, not 13) ----
        pk32 = wp.tile([128, 262], DT, tag="pk32")
        pk16 = wp.tile([128, 3200], F16, tag="pk16")

        def emit_small_weights():
            nc.sync.dma_start(pk32[:], tn["pk32"][:])
            nc.sync.dma_start(pk16[:], tn["pk16"][:])

        def wt(name):
            m32 = {"eye32": (0, 32, 0, 32), "eye64": (0, 64, 0, 64),
                   "pw": (0, 64, 64, 128), "kvb": (0, 128, 128, 129),
                   "b1a": (0, 128, 129, 130), "b1b": (0, 128, 130, 131),
                   "bq": (0, 64, 131, 132), "pb": (0, 64, 132, 133),
                   "b2": (0, 64, 133, 134), "wkv32": (0, 64, 134, 262)}
            m16 = {"w2a": (0, 128, 2048, 2112), "w2b": (0, 128, 2112, 2176),
                   "wq": (0, 64, 2176, 2240), "wkv": (0, 64, 2240, 2368),
                   "w1a": (0, 64, 2368, 2496), "w1b": (0, 64, 2496, 2624),
                   "eye16": (0, 64, 3136, 3200)}
            if name in m32:
                r0, r1, c0, c1 = m32[name]
                return pk32[r0:r1, c0:c1]
            r0, r1, c0, c1 = m16[name]
            return pk16[r0:r1, c0:c1]

        def selh_bm(idx):
            return pk16[0:32, idx * 128:(idx + 1) * 128]

        # ---- persistent mid-size tiles ----
        h_sb = mp.tile([32, T], DT, tag="h_sb")       # row b*8+i, col a*64+s
        hT = mp.tile([64, TLOC], F16, tag="hT")       # row s, col b*256+a*8+i
        y1 = mp.tile([64, TLOC], F16, tag="y1")
        eT = mp.tile([64, TLOC], DT, tag="eT")
        e2 = mp.tile([64, TLOC], F16, tag="e2")
        zp = mp.tile([64, B], DT, tag="zp")
        h2h = mp.tile([32, T], F16, tag="h2h")        # final h, i-major
        # pre-zero: pipelined booster SEL matmuls read not-yet-written rows
        # (weighted 0); garbage fp16 there would turn 0*NaN into NaN.
        nc.vector.memset(h2h[:], 0.0)

        # hT col = b*256 + i*32 + a  (i-major inside each batch block)
        hT4 = hT[:].rearrange("s (b i a) -> s b i a", b=B, i=IPC)

        # channel-dim layer norm (affine folded into downstream weights):
        # y_out = (h - mean) * rsqrt(var + eps), over W token columns.
        # Split into stats/apply so two column groups can pipeline.
        def ln_stats(h_ap, W, lnp, pp, tag):
            sq = lnp.tile([64, W], F16, tag=f"ln_sq_{tag}")
            nc.vector.tensor_mul(sq[:], h_ap, h_ap)
            st = pp.tile([1, 2 * W], DT, tag=f"ln_st_{tag}")
            for c in range(0, W, 512):
                sl = slice(c, min(c + 512, W))
                slm = slice(W + c, W + min(c + 512, W))
                nc.tensor.matmul(st[:, sl], ones64[:, 0:1], h_ap[:, sl],
                                 start=True, stop=True)
                nc.tensor.matmul(st[:, slm], ones64[:, 1:2], sq[:, sl],
                                 start=True, stop=True)
            mean_h = lnp.tile([1, W], F16, tag=f"ln_mh_{tag}")
            nc.vector.tensor_copy(mean_h[:], st[:, 0:W])
            mbsq = lnp.tile([1, W], DT, tag=f"ln_mbsq_{tag}")
            nc.vector.tensor_mul(mbsq[:], st[:, 0:W], mean_h[:])
            var = lnp.tile([1, W], DT, tag=f"ln_var_{tag}")
            nc.vector.scalar_tensor_tensor(var[:], st[:, W:2 * W],
                                           epsv[0:1, 0:1], mbsq[:],
                                           op0=A.add, op1=A.subtract)
            rvar = lnp.tile([1, W], DT, tag=f"ln_rvar_{tag}")
            nc.vector.reciprocal_approx_fast(rvar[:], var[:])
            rstd_h = lnp.tile([1, W], F16, tag=f"ln_rsh_{tag}")
            nc.scalar.sqrt(rstd_h[:], rvar[:])
            return mean_h, rstd_h

        def ln_apply(y_out, h_ap, stats, W, lnp, pp, tag):
            mean_h, rstd_h = stats
            mrb = pp.tile([128, W], DT, tag=f"ln_mrb_{tag}")
            for c in range(0, W, 512):
                sl = slice(c, min(c + 512, W))
                nc.tensor.matmul(mrb[0:64, sl], ones1r[:],
                                 mean_h[:, sl], start=True, stop=True)
                nc.tensor.matmul(mrb[64:128, sl], ones1r[:],
                                 rstd_h[:, sl], start=True, stop=True)
            d = lnp.tile([64, W], DT, tag=f"ln_d_{tag}")
            nc.vector.tensor_sub(d[:], h_ap, mrb[0:64, :])
            nc.vector.tensor_mul(y_out, d[:], mrb[64:128, :])

        def layer_norm(y_out, h_ap, W, lnp, tag):
            with tc.tile_pool(name=f"ps_{tag}", bufs=1, space="PSUM") as pp:
                stats = ln_stats(h_ap, W, lnp, pp, tag)
                ln_apply(y_out, h_ap, stats, W, lnp, pp, tag)

        # ======== Phase A: stream x in t'-chunks; feebler + transpose.
        # Chunks 0-2 are 512 wide; the last 512 is split in two so less
        # work trails the final x bytes.
        CH = CHH
        with nc.named_scope("feebler"), \
             tc.tile_pool(name="fw", bufs=1) as fwp, \
             tc.tile_pool(name="xin", bufs=3) as xp, \
             tc.tile_pool(name="prod", bufs=4) as prp, \
             tc.tile_pool(name="psA", bufs=1, space="PSUM") as psA:
            # fwt holds chunk-major fw: [p, (chunk, m, t_chunk)]
            fwt = fwp.tile([128, 4 * T], F16, tag="fwt")
            for ci, (c0, cw) in enumerate(CH):
                csl = slice(c0, c0 + cw)
                # fw chunk then x chunk: first compute unblocks after ~3MB
                foff = 4 * c0
                nc.sync.dma_start(fwt[:, foff:foff + 4 * cw],
                                  tn["fw"][:, foff:foff + 4 * cw])
                xs = xp.tile([128, B * 4 * 512], F16, tag="xs")
                xoff = B * 4 * c0
                nc.sync.dma_start(xs[:, 0:B * 4 * cw],
                                  tn["x"][:, xoff:xoff + B * 4 * cw])
                if ci == 0:
                    emit_small_weights()
                h_ps = psA.tile([32, 512], DT, tag="h_ps", bufs=2)
                prods = []
                for b in range(B):
                    prod = prp.tile([128, 4 * 512], F16, tag="prod")
                    nc.vector.tensor_mul(
                        prod[:, 0:4 * cw],
                        xs[:, b * 4 * cw:(b + 1) * 4 * cw],
                        fwt[:, 4 * c0:4 * c0 + 4 * cw])
                    prods.append(prod)
                for b in range(B):
                    for m in range(4):
                        o32 = pk16[:, 2624 + (b * 4 + m) * 32:
                                   2624 + (b * 4 + m + 1) * 32]
                        nc.tensor.matmul(
                            h_ps[:, 0:cw], o32,
                            prods[b][:, m * cw:(m + 1) * cw],
                            start=(b == 0 and m == 0),
                            stop=(b == 3 and m == 3))
                nc.scalar.copy(h_sb[:, csl], h_ps[:, 0:cw])
                # transposes for finished 128-col chunks of this chunk
                for cc in range(cw // 128):
                    cidx = (c0 // 128) + cc
                    col = cidx * 128
                    tp = psA.tile([128, 32], DT, tag="tp", bufs=2)
                    nc.tensor.transpose(tp[:], h_sb[:, col:col + 128],
                                        wt("eye32"))
                    for da in range(2):
                        nc.scalar.copy(
                            hT4[:, :, :, 2 * cidx + da],
                            tp[da * 64:(da + 1) * 64, :].rearrange(
                                "s (b i) -> s b i", b=B))

        if "dbg_h" in tn:
            nc.sync.dma_start(tn["dbg_h"][:], h_sb[:])
            nc.sync.dma_start(tn["dbg_hT"][:], hT[:])

        # prefetch booster weights into the post-stream DMA gap
        bwt = bwp.tile([128, 4 * T], F16, tag="bwt")
        nc.sync.dma_start(bwt[:], tn["bw"][:])

        # ======== LN1 -> y1; kv partials; AR1; exp; AR2
        cc1i = dp.tile([64, B], DT, tag="cc1i")
        cc1o = dp.tile([512, B], DT, tag="cc1o", addr_space="Shared")
        cc2i = dp.tile([64, B], DT, tag="cc2i")
        cc2o = dp.tile([512, B], DT, tag="cc2o", addr_space="Shared")
        GW1 = 512
        g1sl = [slice(0, GW1), slice(GW1, 2 * GW1)]
        with nc.named_scope("attn"), \
             tc.tile_pool(name="ln1t", bufs=1) as lnp1:
            with tc.tile_pool(name="psL1", bufs=1, space="PSUM") as pl1:
                sts1 = [ln_stats(hT[:, g1sl[g]], GW1, lnp1, pl1, f"l1{g}")
                        for g in range(2)]
                for g in range(2):
                    ln_apply(y1[:, g1sl[g]], hT[:, g1sl[g]], sts1[g],
                             GW1, lnp1, pl1, f"l1{g}")
            part = lnp1.tile([64, B], DT, tag="part")
            nc.vector.tensor_reduce(
                part[:], y1[:].rearrange("s (b t) -> s b t", b=B),
                axis=X, op=A.add)
            nc.sync.dma_start(cc1i[:], part[:])
            nc.gpsimd.collective_compute("AllGather", A.bypass,
                                         ins=[cc1i[:]], outs=[cc1o[:]],
                                         replica_groups=RG)
            with tc.tile_pool(name="psL", bufs=1, space="PSUM") as psL:
                # q overlaps the AllGather
                q_ps = psL.tile([64, TLOC], DT, tag="q_ps")
                for u in range(2):
                    sl = slice(u * 512, (u + 1) * 512)
                    nc.tensor.matmul(q_ps[:, sl], wt("wq"),
                                     y1[:, sl], start=True, stop=True)
                # dummy exp switches the act table while the AG is in flight
                nc.scalar.activation(warm[:], warm[:], AF.Exp)
                gath = lnp1.tile([64, 8 * B], DT, tag="gath")
                nc.sync.dma_start(
                    gath[:].rearrange("p (s r) -> p s r", s=B),
                    cc1o[:].rearrange("(r p) s -> p s r", r=N_CORES))
                ysum = lnp1.tile([64, B], DT, tag="ysum")
                nc.vector.tensor_reduce(
                    ysum[:], gath[:].rearrange("p (s r) -> p s r", s=B),
                    axis=X, op=A.add)
                kv_ps = psL.tile([128, B], DT, tag="kv_ps")
                nc.tensor.matmul(kv_ps[:], wt("wkv32"),
                                 ysum[:], start=True, stop=True)
                kvgb = lnp1.tile([128, B], DT, tag="kvgb")
                nc.vector.tensor_scalar_add(kvgb[:], kv_ps[:], wt("kvb"))
                ebias = lnp1.tile([64, B], DT, tag="ebias")
                nc.vector.tensor_scalar(ebias[:], kvgb[0:64, :],
                                        wt("bq"), -ESHIFT,
                                        op0=A.mult, op1=A.add)
                for b in range(B):
                    sl = slice(b * 256, (b + 1) * 256)
                    nc.scalar.activation(eT[:, sl], q_ps[:, sl], AF.Exp,
                                         bias=ebias[:, b:b + 1],
                                         scale=kvgb[0:64, b:b + 1],
                                         accum_out=zp[:, b:b + 1])
                nc.sync.dma_start(cc2i[:], zp[:])
                nc.gpsimd.collective_compute("AllGather", A.bypass,
                                             ins=[cc2i[:]], outs=[cc2o[:]],
                                             replica_groups=RG)
                # dummy sqrt flips the act table for LN2 during the AG
                nc.scalar.activation(warm[:], warm[:], AF.Sqrt)
                # cv-scaled projection weights only need AG1's output:
                # compute them inside the AG2 window.
                cvb = lnp1.tile([64, B], DT, tag="cvb")
                nc.vector.tensor_copy(cvb[:], kvgb[64:128, :])
                pwcv = lnp1.tile([64, 4 * SD], F16, tag="pwcv")
                for b in range(B):
                    nc.vector.tensor_scalar_mul(
                        pwcv[:, b * SD:(b + 1) * SD], wt("pw"),
                        cvb[:, b:b + 1])
                gath2 = lnp1.tile([64, 4 * N_CORES], DT, tag="gath2")
                nc.sync.dma_start(
                    gath2[:].rearrange("p (s r) -> p s r", s=B),
                    cc2o[:].rearrange("(r p) s -> p s r", r=N_CORES))
                zg = lnp1.tile([64, B], DT, tag="zg")
                nc.vector.tensor_reduce(
                    zg[:], gath2[:].rearrange("p (s r) -> p s r", s=B),
                    axis=X, op=A.add)
                rz = lnp1.tile([64, B], DT, tag="rz")
                nc.vector.reciprocal_approx_fast(rz[:], zg[:])
                for b in range(B):
                    sl = slice(b * 256, (b + 1) * 256)
                    nc.vector.tensor_scalar_mul(e2[:, sl], eT[:, sl],
                                                rz[:, b:b + 1])
                if "dbg_y1" in tn:
                    nc.sync.dma_start(tn["dbg_y1"][:], y1[:])
                    nc.sync.dma_start(tn["dbg_e2"][:], e2[:])
                    nc.sync.dma_start(tn["dbg_kv"][:, 0:B], kvgb[:])
                    nc.sync.dma_start(tn["dbg_kv"][0:64, B:2 * B], zp[:])
                    nc.sync.dma_start(tn["dbg_kv"][0:64, 2 * B:3 * B], zg[:])

        # ======== proj -> LN2 -> FFN, two pipelined column groups (b01/b23)
        GW = 512
        gsl = [slice(0, GW), slice(GW, 2 * GW)]
        with nc.named_scope("midp"), \
             tc.tile_pool(name="pot", bufs=1) as pot:
            y2 = pot.tile([64, TLOC], F16, tag="y2")
            with tc.tile_pool(name="psM1", bufs=1, space="PSUM") as pm1:
                pj = pm1.tile([64, TLOC], DT, tag="pj")
                for b in range(B):
                    sl = slice(b * 256, (b + 1) * 256)
                    nc.tensor.matmul(pj[:, sl],
                                     pwcv[:, b * SD:(b + 1) * SD],
                                     e2[:, sl], start=True, stop=True)
                for g in range(2):
                    nc.vector.scalar_tensor_tensor(hT[:, gsl[g]],
                                                   pj[:, gsl[g]],
                                                   wt("pb"), hT[:, gsl[g]],
                                                   op0=A.add, op1=A.add)
            with tc.tile_pool(name="psL2", bufs=1, space="PSUM") as pl2:
                stats = [ln_stats(hT[:, gsl[g]], GW, pot, pl2, f"l2{g}")
                         for g in range(2)]
                for g in range(2):
                    ln_apply(y2[:, gsl[g]], hT[:, gsl[g]], stats[g],
                             GW, pot, pl2, f"l2{g}")
            with tc.tile_pool(name="psM2", bufs=1, space="PSUM") as pm2:
                f1as, f1bs, r1as, r1bs = [], [], [], []
                for g in range(2):
                    f1a = pm2.tile([128, GW], DT, tag="f1a", bufs=2)
                    f1b = pm2.tile([128, GW], DT, tag="f1b", bufs=2)
                    nc.tensor.matmul(f1a[:], wt("w1a"), y2[:, gsl[g]],
                                     start=True, stop=True)
                    nc.tensor.matmul(f1b[:], wt("w1b"), y2[:, gsl[g]],
                                     start=True, stop=True)
                    f1as.append(f1a)
                    f1bs.append(f1b)
                for g in range(2):
                    r1a = pot.tile([128, GW], F16, tag="r1a", bufs=2)
                    r1b = pot.tile([128, GW], F16, tag="r1b", bufs=2)
                    nc.scalar.activation(r1a[:], f1as[g][:], AF.Relu,
                                         bias=wt("b1a"))
                    nc.scalar.activation(r1b[:], f1bs[g][:], AF.Relu,
                                         bias=wt("b1b"))
                    r1as.append(r1a)
                    r1bs.append(r1b)
                f2s = []
                for g in range(2):
                    f2 = pm2.tile([64, GW], DT, tag="f2", bufs=2)
                    nc.tensor.matmul(f2[:], wt("w2a"), r1as[g][:],
                                     start=True, stop=False)
                    nc.tensor.matmul(f2[:], wt("w2b"), r1bs[g][:],
                                     start=False, stop=True)
                    f2s.append(f2)
                for g in range(2):
                    nc.vector.scalar_tensor_tensor(hT[:, gsl[g]], f2s[g][:],
                                                   wt("b2"), hT[:, gsl[g]],
                                                   op0=A.add, op1=A.add)

        # ======== Booster: back-transpose h (DRAM fold) per batch, then
        # per (b,m): SEL-broadcast matmul, DVE/GpSimd multiply straight
        # from PSUM, stream out.
        if "dbg_h3" in tn:
            nc.sync.dma_start(tn["dbg_h3"][:], hT[:])
        hr_d = dp.tile([32, T], F16, tag="hr_d")
        with nc.named_scope("booster"), \
             tc.tile_pool(name="bst", bufs=1) as bst, \
             tc.tile_pool(name="psB", bufs=1, space="PSUM") as psB:
            eye64 = wt("eye64")

            def trfold(b):
                for cq in range(2):
                    tpb = psB.tile([128, 64], F16, tag="tpb", bufs=2)
                    col = b * 256 + cq * 128
                    nc.tensor.transpose(tpb[:], hT[:, col:col + 128],
                                        wt("eye16"))
                    stage = bst.tile([128, 64], F16, tag="stage", bufs=2)
                    nc.scalar.copy(stage[:], tpb[:])
                    # chunk cols = (i_rel 4, a 32); each i gives one full
                    # 2048-wide hr_d row.
                    r0 = b * 8 + cq * 4
                    nc.sync.dma_start(
                        hr_d[r0:r0 + 4, :].rearrange("i (a s) -> (i a) s",
                                                     a=32),
                        stage[:])
                nc.sync.dma_start(h2h[b * 8:b * 8 + 8, :],
                                  hr_d[b * 8:b * 8 + 8, :])

            def chunks(b):
                for m in range(4):
                    pr = bst.tile([128, T], F16, tag="pr", bufs=3)
                    mode = ("dve", "sc_dve", "dve", "sc_gp")[m]
                    for half in range(2):
                        bc = psB.tile([128, 1024], DT, tag="bc", bufs=3)
                        hsl = slice(half * 1024, (half + 1) * 1024)
                        for u in range(2):
                            us = slice(half * 1024 + u * 512,
                                       half * 1024 + (u + 1) * 512)
                            nc.tensor.matmul(
                                bc[:, u * 512:(u + 1) * 512],
                                selh_bm(b * 4 + m),
                                h2h[:, us], start=True, stop=True)
                        bsl = bwt[:, m * T + half * 1024:
                                  m * T + (half + 1) * 1024]
                        if mode == "dve":
                            # DVE reads the broadcast straight from PSUM
                            nc.vector.tensor_mul(pr[:, hsl], bsl, bc[:])
                        else:
                            # scalar copy frees the PSUM slot quickly;
                            # the 16-bit SBUF multiply then runs cheaper
                            bch = bst.tile([128, 1024], F16, tag="bch",
                                           bufs=3)
                            nc.scalar.copy(bch[:], bc[:])
                            eng = (nc.vector if mode == "sc_dve"
                                   else nc.gpsimd)
                            eng.tensor_mul(pr[:, hsl], bsl, bch[:])
                    r0 = (b * 8 + 2 * m) * 64
                    nc.sync.dma_start(out[r0:r0 + 128, :], pr[:])

            # software pipeline: fold batch b+1 while b's chunks stream
            trfold(0)
            trfold(1)
            chunks(0)
            trfold(2)
            chunks(1)
            trfold(3)
            chunks(2)
            chunks(3)
            if "dbg_h2h" in tn:
                nc.sync.dma_start(tn["dbg_h2h"][:], h2h[:])


def _prep_host(inputs):
    """Host-side prep: shard x/fw/bw per core (fp16); fold LN affines into
    the downstream weights; pack small weights."""
    f32 = np.float32
    g = {k: np.asarray(v, dtype=f32) for k, v in inputs.items()}
    x = g["x"].reshape(B, SD, SD, T)          # flat view (b, i, j, t')
    fw, bw = g["feebler_w"], g["booster_w"]
    wq, wk, wv = g["wq"], g["wk"], g["wv"]
    wqkv = np.concatenate([w.transpose(1, 0, 2).reshape(SD, SD)
                           for w in (wq, wk, wv)], axis=1)  # [64, 192]
    # fold ln1 gamma into wqkv rows; ln1 beta becomes additive biases
    g1 = g["ln1_g"].reshape(SD, 1)
    wqkv_g = wqkv * g1
    bqv = g["ln1_b"] @ wqkv[:, 0:64] * 1.0          # [64] q bias
    bk = g["ln1_b"] @ wqkv[:, 64:128]
    bv = g["ln1_b"] @ wqkv[:, 128:192]
    kvb = np.concatenate([bk, bv]) * float(T)       # [128] k/v sum bias
    # fold ln2 gamma into w1 rows; ln2 beta into b1
    g2 = g["ln2_g"].reshape(SD, 1)
    w1_g = g["w1"] * g2
    b1f = g["b1"] + g["ln2_b"] @ g["w1"]
    b1h = b1f.reshape(2, 128).T.astype(f32)     # [128, 2]
    pk32 = np.zeros((128, 262), f32)
    pk32[0:64, 0:64] = np.eye(64, dtype=f32)
    pk32[0:64, 64:128] = g["proj_w"]
    pk32[:, 128] = kvb
    pk32[:, 129:131] = b1h
    pk32[0:64, 131] = bqv
    pk32[0:64, 132] = g["proj_b"]
    pk32[0:64, 133] = g["b2"]
    pk32[0:64, 134:262] = wqkv_g[:, 64:192]
    sel = np.zeros((32, 2048), np.float16)
    for b in range(B):
        for m in range(4):
            c0 = (b * 4 + m) * 128
            sel[b * 8 + 2 * m, c0:c0 + 64] = 1.0
            sel[b * 8 + 2 * m + 1, c0 + 64:c0 + 128] = 1.0
    pk16 = np.zeros((128, 3200), np.float16)
    pk16[0:64, 3136:3200] = np.eye(64, dtype=np.float16)
    pk16[0:32, 0:2048] = sel
    pk16[:, 2048:2112] = g["w2"][0:128, :].astype(np.float16)
    pk16[:, 2112:2176] = g["w2"][128:256, :].astype(np.float16)
    pk16[0:64, 2176:2368] = wqkv_g.astype(np.float16)
    pk16[0:64, 2368:2624] = w1_g.astype(np.float16)
    for b in range(B):
        for m in range(4):
            c0 = 2624 + (b * 4 + m) * 32
            pk16[0:64, c0 + b * 8 + 2 * m] = 1.0
            pk16[64:128, c0 + b * 8 + 2 * m + 1] = 1.0
    shared = {"pk32": pk32, "pk16": np.ascontiguousarray(pk16)}
    in_maps = []
    for k in range(N_CORES):
        i0 = k * IPC
        m = dict(shared)
        # x: rows (b,i,j) -> [p=(i%2,j), (chunk, b, q=i//2, t_chunk)]
        xv = x[:, i0:i0 + IPC].reshape(B, 4, 128, T).astype(np.float16)
        m["x"] = np.concatenate(
            [xv[:, :, :, c0:c0 + cw].transpose(2, 0, 1, 3).reshape(128, -1)
             for c0, cw in CHH], axis=1)
        # fw: [p, (chunk, m, t_chunk)]
        fv = fw[i0:i0 + IPC].reshape(4, 128, T).astype(np.float16)
        m["fw"] = np.concatenate(
            [fv[:, :, c0:c0 + cw].transpose(1, 0, 2).reshape(128, -1)
             for c0, cw in CHH], axis=1)
        # booster output is sharded over j (rev[b,i,j]=bw[i,j]*hr[b,j]):
        # rows (j_loc, i) so the broadcast h row per 64-row group is local
        m["bw"] = np.ascontiguousarray(
            bw[:, i0:i0 + IPC].transpose(1, 0, 2).reshape(
                4, 128, T).transpose(1, 0, 2).reshape(
                128, 4 * T)).astype(np.float16)
        in_maps.append(m)
    return in_maps


def _get_nc():
    if "nc" not in _CACHE:
        _CACHE["nc"] = _build_nc()
    return _CACHE["nc"]


def run(inputs, trace=False, **kw):
    nc = _get_nc()
    in_maps = _prep_host(inputs)
    res = run_bass_kernel_spmd(nc, in_maps, core_ids=list(range(N_CORES)),
                               trace=trace, **kw)
    full = np.empty((B, SD, SD, T), dtype=np.float32)
    for k in range(N_CORES):
        i0 = k * IPC
        co = res.results[k]["out"].astype(np.float32).reshape(B, IPC, SD, T)
        full[:, :, i0:i0 + IPC] = co.transpose(0, 2, 1, 3)
    return full.reshape(B, T, NE), res


def kernel(**inputs):
    out, _ = run(inputs)
    return out


# revision 46
# speedup vs baseline: 1.0133x; 1.0133x over previous
"""Trainium2 Bass kernel for nn_Block_69191923139027 (dense_transformer).

Sharding: 8 cores; core k owns Feebler/Booster rows i in [8k, 8k+8), which
is exactly tokens [256k, 256k+256) per batch, so the Booster needs only
locally-computed h (no h AllGather). Two tiny AllReduces stitch the global
k/v sums and softmax denominators.

v5: quarter-pipelined x stream keeps PE warm (2.4GHz), constant [128,2]
feebler stationary, AllReduce of pre-projected k/v partials, preloaded
activation tables, 16-bit stationaries/moving operands everywhere,
softmax 1/z applied to e (bounded <=1) instead of the proj weights,
booster multiplies read PSUM directly split across DVE and GpSimd.

Self-contained: hardcodes all shapes; no sibling imports.
"""

import numpy as np

import concourse.bacc as bacc
import concourse.mybir as mybir
import concourse.tile as tile
from concourse.bass_utils import run_bass_kernel_spmd

N_CORES = 8
B, T, SD, NE = 4, 2048, 64, 4096
H, HS, FH = 8, 8, 256
EPS = 1e-5
IPC = SD // N_CORES          # 8 feebler rows per core
TLOC = B * IPC * 32          # 1024 local tokens; hT col = b*256 + a*8 + i
DT = mybir.dt.float32
F16 = mybir.dt.float16
RG = [list(range(N_CORES))]
ESHIFT = 64.0                # softmax logit shift (max |logit| ~ 71)

_CACHE = {}
CHH = [(0, 512), (512, 512), (1024, 512), (1536, 256), (1792, 256)]


def _build_nc():
    nc = bacc.Bacc("TRN2", target_bir_lowering=False, debug=False,
                   num_devices=N_CORES)
    A = mybir.AluOpType
    AF = mybir.ActivationFunctionType

    tn = {}
    # x/fw/bw are host-pretransposed to partition-major so every stream
    # DMA is a 2D contiguous copy (128 descriptors, not 2048).
    tn["x"] = nc.dram_tensor("x", [128, B * 4 * T], F16,
                             kind="ExternalInput")
    tn["fw"] = nc.dram_tensor("fw", [128, 4 * T], F16, kind="ExternalInput")
    tn["bw"] = nc.dram_tensor("bw", [128, 4 * T], F16, kind="ExternalInput")
    # all small weights packed into two tensors (2 DMA issues, not 13)
    tn["pk32"] = nc.dram_tensor("pk32", [128, 262], DT, kind="ExternalInput")
    tn["pk16"] = nc.dram_tensor("pk16", [128, 3200], F16,
                                kind="ExternalInput")
    out = nc.dram_tensor("out", [B * IPC * SD, T], F16, kind="ExternalOutput")
    if _CACHE.get("debug"):
        tn["dbg_h"] = nc.dram_tensor("dbg_h", [32, T], DT,
                                     kind="ExternalOutput")
        tn["dbg_hT"] = nc.dram_tensor("dbg_hT", [64, TLOC], F16,
                                      kind="ExternalOutput")
        tn["dbg_y1"] = nc.dram_tensor("dbg_y1", [64, TLOC], F16,
                                      kind="ExternalOutput")
        tn["dbg_e2"] = nc.dram_tensor("dbg_e2", [64, TLOC], F16,
                                      kind="ExternalOutput")
        tn["dbg_kv"] = nc.dram_tensor("dbg_kv", [128, 3 * B], DT,
                                      kind="ExternalOutput")
        tn["dbg_h3"] = nc.dram_tensor("dbg_h3", [64, TLOC], F16,
                                      kind="ExternalOutput")
        tn["dbg_h2h"] = nc.dram_tensor("dbg_h2h", [32, T], F16,
                                       kind="ExternalOutput")

    with tile.TileContext(nc) as tc:
        _body(nc, tc, tn, out, A, AF)
    nc.compile()
    return nc


def _body(nc, tc, tn, out, A, AF):
    X = mybir.AxisListType.X

    with tc.tile_pool(name="wconst", bufs=1) as wp, \
         tc.tile_pool(name="mid", bufs=1) as mp, \
         tc.tile_pool(name="bwpool", bufs=1) as bwp, \
         tc.tile_pool(name="dram", bufs=1, space="DRAM") as dp:

        # ---- on-chip constants ----
        ones64 = wp.tile([SD, 2], F16, tag="ones64")
        nc.vector.memset(ones64[:], 1.0 / SD)
        ones1r = wp.tile([1, SD], F16, tag="ones1r")
        nc.vector.memset(ones1r[:], 1.0)
        epsv = wp.tile([1, 1], DT, tag="epsv")
        nc.vector.memset(epsv[:], EPS)

        # preload activation tables (Exp/Sqrt/Square/Relu/Identity) so no
        # ACT_TABLE_LOAD lands on the critical path later.
        warm = wp.tile([1, 2], DT, tag="warm")
        nc.vector.memset(warm[:], 1.0)
        for fn in (AF.Identity, AF.Square, AF.Sqrt, AF.Relu, AF.Exp):
            nc.scalar.activation(warm[:], warm[:], fn)

        # warm-up collective: absorbs the one-time CC setup + barrier
        # while the feebler streams.
        ccw_i = dp.tile([1, 1], DT, tag="ccw_i")
        ccw_o = dp.tile([N_CORES, 1], DT, tag="ccw_o", addr_space="Shared")
        warm1 = wp.tile([1, 1], DT, tag="warm1")
        nc.vector.memset(warm1[:], 0.0)
        nc.sync.dma_start(ccw_i[:], warm1[:])
        nc.gpsimd.collective_compute("AllGather", A.bypass,
                                     ins=[ccw_i[:]], outs=[ccw_o[:]],
                                     replica_groups=RG)

        # ---- small weights, two packed tiles (2 DMA issues, not 13) ----
        pk32 = wp.tile([128, 262], DT, tag="pk32")
        pk16 = wp.tile([128, 3200], F16, tag="pk16")

        def emit_small_weights():
            nc.sync.dma_start(pk32[:], tn["pk32"][:])
            nc.sync.dma_start(pk16[:], tn["pk16"][:])

        def wt(name):
            m32 = {"eye32": (0, 32, 0, 32), "eye64": (0, 64, 0, 64),
                   "pw": (0, 64, 64, 128), "kvb": (0, 128, 128, 129),
                   "b1a": (0, 128, 129, 130), "b1b": (0, 128, 130, 131),
                   "bq": (0, 64, 131, 132), "pb": (0, 64, 132, 133),
                   "b2": (0, 64, 133, 134), "wkv32": (0, 64, 134, 262)}
            m16 = {"w2a": (0, 128, 2048, 2112), "w2b": (0, 128, 2112, 2176),
                   "wq": (0, 64, 2176, 2240), "wkv": (0, 64, 2240, 2368),
                   "w1a": (0, 64, 2368, 2496), "w1b": (0, 64, 2496, 2624),
                   "eye16": (0, 64, 3136, 3200)}
            if name in m32:
                r0, r1, c0, c1 = m32[name]
                return pk32[r0:r1, c0:c1]
            r0, r1, c0, c1 = m16[name]
            return pk16[r0:r1, c0:c1]

        def selh_bm(idx):
            return pk16[0:32, idx * 128:(idx + 1) * 128]

        # ---- persistent mid-size tiles ----
        h_sb = mp.tile([32, T], DT, tag="h_sb")       # row b*8+i, col a*64+s
        hT = mp.tile([64, TLOC], F16, tag="hT")       # row s, col b*256+a*8+i
        y1 = mp.tile([64, TLOC], F16, tag="y1")
        eT = mp.tile([64, TLOC], DT, tag="eT")
        e2 = mp.tile([64, TLOC], F16, tag="e2")
        zp = mp.tile([64, B], DT, tag="zp")
        h2h = mp.tile([32, T], F16, tag="h2h")        # final h, i-major
        # pre-zero: pipelined booster SEL matmuls read not-yet-written rows
        # (weighted 0); garbage fp16 there would turn 0*NaN into NaN.
        nc.vector.memset(h2h[:], 0.0)

        # hT col = b*256 + i*32 + a  (i-major inside each batch block)
        hT4 = hT[:].rearrange("s (b i a) -> s b i a", b=B, i=IPC)

        # channel-dim layer norm (affine folded into downstream weights):
        # y_out = (h - mean) * rsqrt(var + eps), over W token columns.
        # Split into stats/apply so two column groups can pipeline.
        def ln_stats(h_ap, W, lnp, pp, tag):
            sq = lnp.tile([64, W], F16, tag=f"ln_sq_{tag}")
            nc.vector.tensor_mul(sq[:], h_ap, h_ap)
            st = pp.tile([1, 2 * W], DT, tag=f"ln_st_{tag}")
            for c in range(0, W, 512):
                sl = slice(c, min(c + 512, W))
                slm = slice(W + c, W + min(c + 512, W))
                nc.tensor.matmul(st[:, sl], ones64[:, 0:1], h_ap[:, sl],
                                 start=True, stop=True)
                nc.tensor.matmul(st[:, slm], ones64[:, 1:2], sq[:, sl],
                                 start=True, stop=True)
            mean_h = lnp.tile([1, W], F16, tag=f"ln_mh_{tag}")
            nc.vector.tensor_copy(mean_h[:], st[:, 0:W])
            mbsq = lnp.tile([1, W], DT, tag=f"ln_mbsq_{tag}")
            nc.vector.tensor_mul(mbsq[:], st[:, 0:W], mean_h[:])
            var = lnp.tile([1, W], DT, tag=f"ln_var_{tag}")
            nc.vector.scalar_tensor_tensor(var[:], st[:, W:2 * W],
                                           epsv[0:1, 0:1], mbsq[:],
                                           op0=A.add, op1=A.subtract)
            rvar = lnp.tile([1, W], DT, tag=f"ln_rvar_{tag}")
            nc.vector.reciprocal_approx_fast(rvar[:], var[:])
            rstd_h = lnp.tile([1, W], F16, tag=f"ln_rsh_{tag}")
            nc.scalar.sqrt(rstd_h[:], rvar[:])
            return mean_h, rstd_h

        def ln_apply(y_out, h_ap, stats, W, lnp, pp, tag):
            mean_h, rstd_h = stats
            mrb = pp.tile([128, W], DT, tag=f"ln_mrb_{tag}")
            for c in range(0, W, 512):
                sl = slice(c, min(c + 512, W))
                nc.tensor.matmul(mrb[0:64, sl], ones1r[:],
                                 mean_h[:, sl], start=True, stop=True)
                nc.tensor.matmul(mrb[64:128, sl], ones1r[:],
                                 rstd_h[:, sl], start=True, stop=True)
            d = lnp.tile([64, W], DT, tag=f"ln_d_{tag}")
            nc.vector.tensor_sub(d[:], h_ap, mrb[0:64, :])
            nc.vector.tensor_mul(y_out, d[:], mrb[64:128, :])

        def layer_norm(y_out, h_ap, W, lnp, tag):
            with tc.tile_pool(name=f"ps_{tag}", bufs=1, space="PSUM") as pp:
                stats = ln_stats(h_ap, W, lnp, pp, tag)
                ln_apply(y_out, h_ap, stats, W, lnp, pp, tag)

        # ======== Phase A: stream x in t'-chunks; feebler + transpose.
        # Chunks 0-2 are 512 wide; the last 512 is split in two so less
        # work trails the final x bytes.
        CH = CHH
        with nc.named_scope("feebler"), \
             tc.tile_pool(name="fw", bufs=1) as fwp, \
             tc.tile_pool(name="xin", bufs=2) as xp, \
             tc.tile_pool(name="prod", bufs=4) as prp, \
             tc.tile_pool(name="psA", bufs=1, space="PSUM") as psA:
            # fwt holds chunk-major fw: [p, (chunk, m, t_chunk)]
            fwt = fwp.tile([128, 4 * T], F16, tag="fwt")
            for ci, (c0, cw) in enumerate(CH):
                csl = slice(c0, c0 + cw)
                # fw chunk then x chunk: first compute unblocks after ~3MB
                foff = 4 * c0
                nc.sync.dma_start(fwt[:, foff:foff + 4 * cw],
                                  tn["fw"][:, foff:foff + 4 * cw])
                xs = xp.tile([128, B * 4 * 512], F16, tag="xs")
                xoff = B * 4 * c0
                nc.sync.dma_start(xs[:, 0:B * 4 * cw],
                                  tn["x"][:, xoff:xoff + B * 4 * cw])
                if ci == 0:
                    emit_small_weights()
                h_ps = psA.tile([32, 512], DT, tag="h_ps", bufs=2)
                prods = []
                for b in range(B):
                    prod = prp.tile([128, 4 * 512], F16, tag="prod")
                    nc.vector.tensor_mul(
                        prod[:, 0:4 * cw],
                        xs[:, b * 4 * cw:(b + 1) * 4 * cw],
                        fwt[:, 4 * c0:4 * c0 + 4 * cw])
                    prods.append(prod)
                for b in range(B):
                    for m in range(4):
                        o32 = pk16[:, 2624 + (b * 4 + m) * 32:
                                   2624 + (b * 4 + m + 1) * 32]
                        nc.tensor.matmul(
                            h_ps[:, 0:cw], o32,
                            prods[b][:, m * cw:(m + 1) * cw],
                            start=(b == 0 and m == 0),
                            stop=(b == 3 and m == 3))
                nc.scalar.copy(h_sb[:, csl], h_ps[:, 0:cw])
                # transposes for finished 128-col chunks of this chunk
                for cc in range(cw // 128):
                    cidx = (c0 // 128) + cc
                    col = cidx * 128
                    tp = psA.tile([128, 32], DT, tag="tp", bufs=2)
                    nc.tensor.transpose(tp[:], h_sb[:, col:col + 128],
                                        wt("eye32"))
                    for da in range(2):
                        nc.scalar.copy(
                            hT4[:, :, :, 2 * cidx + da],
                            tp[da * 64:(da + 1) * 64, :].rearrange(
                                "s (b i) -> s b i", b=B))

        if "dbg_h" in tn:
            nc.sync.dma_start(tn["dbg_h"][:], h_sb[:])
            nc.sync.dma_start(tn["dbg_hT"][:], hT[:])

        # prefetch booster weights into the post-stream DMA gap
        bwt = bwp.tile([128, 4 * T], F16, tag="bwt")
        nc.sync.dma_start(bwt[:], tn["bw"][:])

        # ======== LN1 -> y1; kv partials; AR1; exp; AR2
        cc1i = dp.tile([64, B], DT, tag="cc1i")
        cc1o = dp.tile([512, B], DT, tag="cc1o", addr_space="Shared")
        cc2i = dp.tile([64, B], DT, tag="cc2i")
        cc2o = dp.tile([512, B], DT, tag="cc2o", addr_space="Shared")
        GW1 = 512
        g1sl = [slice(0, GW1), slice(GW1, 2 * GW1)]
        with nc.named_scope("attn"), \
             tc.tile_pool(name="ln1t", bufs=1) as lnp1:
            with tc.tile_pool(name="psL1", bufs=1, space="PSUM") as pl1:
                sts1 = [ln_stats(hT[:, g1sl[g]], GW1, lnp1, pl1, f"l1{g}")
                        for g in range(2)]
                for g in range(2):
                    ln_apply(y1[:, g1sl[g]], hT[:, g1sl[g]], sts1[g],
                             GW1, lnp1, pl1, f"l1{g}")
            part = lnp1.tile([64, B], DT, tag="part")
            nc.vector.tensor_reduce(
                part[:], y1[:].rearrange("s (b t) -> s b t", b=B),
                axis=X, op=A.add)
            nc.sync.dma_start(cc1i[:], part[:])
            nc.gpsimd.collective_compute("AllGather", A.bypass,
                                         ins=[cc1i[:]], outs=[cc1o[:]],
                                         replica_groups=RG)
            with tc.tile_pool(name="psL", bufs=1, space="PSUM") as psL:
                # q overlaps the AllGather
                q_ps = psL.tile([64, TLOC], DT, tag="q_ps")
                for u in range(2):
                    sl = slice(u * 512, (u + 1) * 512)
                    nc.tensor.matmul(q_ps[:, sl], wt("wq"),
                                     y1[:, sl], start=True, stop=True)
                # dummy exp switches the act table while the AG is in flight
                nc.scalar.activation(warm[:], warm[:], AF.Exp)
                gath = lnp1.tile([64, 8 * B], DT, tag="gath")
                nc.sync.dma_start(
                    gath[:].rearrange("p (s r) -> p s r", s=B),
                    cc1o[:].rearrange("(r p) s -> p s r", r=N_CORES))
                ysum = lnp1.tile([64, B], DT, tag="ysum")
                nc.vector.tensor_reduce(
                    ysum[:], gath[:].rearrange("p (s r) -> p s r", s=B),
                    axis=X, op=A.add)
                kv_ps = psL.tile([128, B], DT, tag="kv_ps")
                nc.tensor.matmul(kv_ps[:], wt("wkv32"),
                                 ysum[:], start=True, stop=True)
                kvgb = lnp1.tile([128, B], DT, tag="kvgb")
                nc.vector.tensor_scalar_add(kvgb[:], kv_ps[:], wt("kvb"))
                ebias = lnp1.tile([64, B], DT, tag="ebias")
                nc.vector.tensor_scalar(ebias[:], kvgb[0:64, :],
                                        wt("bq"), -ESHIFT,
                                        op0=A.mult, op1=A.add)
                for b in range(B):
                    sl = slice(b * 256, (b + 1) * 256)
                    nc.scalar.activation(eT[:, sl], q_ps[:, sl], AF.Exp,
                                         bias=ebias[:, b:b + 1],
                                         scale=kvgb[0:64, b:b + 1],
                                         accum_out=zp[:, b:b + 1])
                nc.sync.dma_start(cc2i[:], zp[:])
                nc.gpsimd.collective_compute("AllGather", A.bypass,
                                             ins=[cc2i[:]], outs=[cc2o[:]],
                                             replica_groups=RG)
                # dummy sqrt flips the act table for LN2 during the AG
                nc.scalar.activation(warm[:], warm[:], AF.Sqrt)
                # cv-scaled projection weights only need AG1's output:
                # compute them inside the AG2 window.
                cvb = lnp1.tile([64, B], DT, tag="cvb")
                nc.vector.tensor_copy(cvb[:], kvgb[64:128, :])
                pwcv = lnp1.tile([64, 4 * SD], F16, tag="pwcv")
                for b in range(B):
                    nc.vector.tensor_scalar_mul(
                        pwcv[:, b * SD:(b + 1) * SD], wt("pw"),
                        cvb[:, b:b + 1])
                gath2 = lnp1.tile([64, 4 * N_CORES], DT, tag="gath2")
                nc.sync.dma_start(
                    gath2[:].rearrange("p (s r) -> p s r", s=B),
                    cc2o[:].rearrange("(r p) s -> p s r", r=N_CORES))
                zg = lnp1.tile([64, B], DT, tag="zg")
                nc.vector.tensor_reduce(
                    zg[:], gath2[:].rearrange("p (s r) -> p s r", s=B),
                    axis=X, op=A.add)
                rz = lnp1.tile([64, B], DT, tag="rz")
                nc.vector.reciprocal_approx_fast(rz[:], zg[:])
                for b in range(B):
                    sl = slice(b * 256, (b + 1) * 256)
                    nc.vector.tensor_scalar_mul(e2[:, sl], eT[:, sl],
                                                rz[:, b:b + 1])
                if "dbg_y1" in tn:
                    nc.sync.dma_start(tn["dbg_y1"][:], y1[:])
                    nc.sync.dma_start(tn["dbg_e2"][:], e2[:])
                    nc.sync.dma_start(tn["dbg_kv"][:, 0:B], kvgb[:])
                    nc.sync.dma_start(tn["dbg_kv"][0:64, B:2 * B], zp[:])
                    nc.sync.dma_start(tn["dbg_kv"][0:64, 2 * B:3 * B], zg[:])

        # ======== proj -> LN2 -> FFN, two pipelined column groups (b01/b23)
        GW = 512
        gsl = [slice(0, GW), slice(GW, 2 * GW)]
        with nc.named_scope("midp"), \
             tc.tile_pool(name="pot", bufs=1) as pot:
            y2 = pot.tile([64, TLOC], F16, tag="y2")
            with tc.tile_pool(name="psM1", bufs=1, space="PSUM") as pm1:
                pj = pm1.tile([64, TLOC], DT, tag="pj")
                for b in range(B):
                    sl = slice(b * 256, (b + 1) * 256)
                    nc.tensor.matmul(pj[:, sl],
                                     pwcv[:, b * SD:(b + 1) * SD],
                                     e2[:, sl], start=True, stop=True)
                for g in range(2):
                    nc.vector.scalar_tensor_tensor(hT[:, gsl[g]],
                                                   pj[:, gsl[g]],
                                                   wt("pb"), hT[:, gsl[g]],
                                                   op0=A.add, op1=A.add)
            with tc.tile_pool(name="psL2", bufs=1, space="PSUM") as pl2:
                stats = [ln_stats(hT[:, gsl[g]], GW, pot, pl2, f"l2{g}")
                         for g in range(2)]
                for g in range(2):
                    ln_apply(y2[:, gsl[g]], hT[:, gsl[g]], stats[g],
                             GW, pot, pl2, f"l2{g}")
            with tc.tile_pool(name="psM2", bufs=1, space="PSUM") as pm2:
                f1as, f1bs, r1as, r1bs = [], [], [], []
                for g in range(2):
                    f1a = pm2.tile([128, GW], DT, tag="f1a", bufs=2)
                    f1b = pm2.tile([128, GW], DT, tag="f1b", bufs=2)
                    nc.tensor.matmul(f1a[:], wt("w1a"), y2[:, gsl[g]],
                                     start=True, stop=True)
                    nc.tensor.matmul(f1b[:], wt("w1b"), y2[:, gsl[g]],
                                     start=True, stop=True)
                    f1as.append(f1a)
                    f1bs.append(f1b)
                for g in range(2):
                    r1a = pot.tile([128, GW], F16, tag="r1a", bufs=2)
                    r1b = pot.tile([128, GW], F16, tag="r1b", bufs=2)
                    nc.scalar.activation(r1a[:], f1as[g][:], AF.Relu,
                                         bias=wt("b1a"))
                    nc.scalar.activation(r1b[:], f1bs[g][:], AF.Relu,
                                         bias=wt("b1b"))
                    r1as.append(r1a)
                    r1bs.append(r1b)
                f2s = []
                for g in range(2):
                    f2 = pm2.tile([64, GW], DT, tag="f2", bufs=2)
                    nc.tensor.matmul(f2[:], wt("w2a"), r1as[g][:],
                                     start=True, stop=False)
                    nc.tensor.matmul(f2[:], wt("w2b"), r1bs[g][:],
                                     start=False, stop=True)
                    f2s.append(f2)
                for g in range(2):
                    nc.vector.scalar_tensor_tensor(hT[:, gsl[g]], f2s[g][:],
                                                   wt("b2"), hT[:, gsl[g]],
                                                   op0=A.add, op1=A.add)

        # ======== Booster: back-transpose h (DRAM fold) per batch, then
        # per (b,m): SEL-broadcast matmul, DVE/GpSimd multiply straight
        # from PSUM, stream out.
        if "dbg_h3" in tn:
            nc.sync.dma_start(tn["dbg_h3"][:], hT[:])
        hr_d = dp.tile([32, T], F16, tag="hr_d")
        with nc.named_scope("booster"), \
             tc.tile_pool(name="bst", bufs=1) as bst, \
             tc.tile_pool(name="psB", bufs=1, space="PSUM") as psB:
            eye64 = wt("eye64")

            def trfold(b):
                for cq in range(2):
                    tpb = psB.tile([128, 64], F16, tag="tpb", bufs=2)
                    col = b * 256 + cq * 128
                    nc.tensor.transpose(tpb[:], hT[:, col:col + 128],
                                        wt("eye16"))
                    stage = bst.tile([128, 64], F16, tag="stage", bufs=2)
                    nc.scalar.copy(stage[:], tpb[:])
                    # chunk cols = (i_rel 4, a 32); each i gives one full
                    # 2048-wide hr_d row.
                    r0 = b * 8 + cq * 4
                    nc.sync.dma_start(
                        hr_d[r0:r0 + 4, :].rearrange("i (a s) -> (i a) s",
                                                     a=32),
                        stage[:])
                nc.sync.dma_start(h2h[b * 8:b * 8 + 8, :],
                                  hr_d[b * 8:b * 8 + 8, :])

            def chunks(b):
                for m in range(4):
                    pr = bst.tile([128, T], F16, tag="pr", bufs=3)
                    mode = ("dve", "sc_dve", "dve", "sc_gp")[m]
                    for half in range(2):
                        bc = psB.tile([128, 1024], DT, tag="bc", bufs=3)
                        hsl = slice(half * 1024, (half + 1) * 1024)
                        for u in range(2):
                            us = slice(half * 1024 + u * 512,
                                       half * 1024 + (u + 1) * 512)
                            nc.tensor.matmul(
                                bc[:, u * 512:(u + 1) * 512],
                                selh_bm(b * 4 + m),
                                h2h[:, us], start=True, stop=True)
                        bsl = bwt[:, m * T + half * 1024:
                                  m * T + (half + 1) * 1024]
                        if mode == "dve":
                            # DVE reads the broadcast straight from PSUM
                            nc.vector.tensor_mul(pr[:, hsl], bsl, bc[:])
                        else:
                            # scalar copy frees the PSUM slot quickly;
                            # the 16-bit SBUF multiply then runs cheaper
                            bch = bst.tile([128, 1024], F16, tag="bch",
                                           bufs=3)
                            nc.scalar.copy(bch[:], bc[:])
                            eng = (nc.vector if mode == "sc_dve"
                                   else nc.gpsimd)
                            eng.tensor_mul(pr[:, hsl], bsl, bch[:])
                    r0 = (b * 8 + 2 * m) * 64
                    nc.sync.dma_start(out[r0:r0 + 128, :], pr[:])

            # software pipeline: fold batch b+1 while b's chunks stream
            trfold(0)
            trfold(1)
            chunks(0)
            trfold(2)
            chunks(1)
            trfold(3)
            chunks(2)
            chunks(3)
            if "dbg_h2h" in tn:
                nc.sync.dma_start(tn["dbg_h2h"][:], h2h[:])


def _prep_host(inputs):
    """Host-side prep: shard x/fw/bw per core (fp16); fold LN affines into
    the downstream weights; pack small weights."""
    f32 = np.float32
    g = {k: np.asarray(v, dtype=f32) for k, v in inputs.items()}
    x = g["x"].reshape(B, SD, SD, T)          # flat view (b, i, j, t')
    fw, bw = g["feebler_w"], g["booster_w"]
    wq, wk, wv = g["wq"], g["wk"], g["wv"]
    wqkv = np.concatenate([w.transpose(1, 0, 2).reshape(SD, SD)
                           for w in (wq, wk, wv)], axis=1)  # [64, 192]
    # fold ln1 gamma into wqkv rows; ln1 beta becomes additive biases
    g1 = g["ln1_g"].reshape(SD, 1)
    wqkv_g = wqkv * g1
    bqv = g["ln1_b"] @ wqkv[:, 0:64] * 1.0          # [64] q bias
    bk = g["ln1_b"] @ wqkv[:, 64:128]
    bv = g["ln1_b"] @ wqkv[:, 128:192]
    kvb = np.concatenate([bk, bv]) * float(T)       # [128] k/v sum bias
    # fold ln2 gamma into w1 rows; ln2 beta into b1
    g2 = g["ln2_g"].reshape(SD, 1)
    w1_g = g["w1"] * g2
    b1f = g["b1"] + g["ln2_b"] @ g["w1"]
    b1h = b1f.reshape(2, 128).T.astype(f32)     # [128, 2]
    pk32 = np.zeros((128, 262), f32)
    pk32[0:64, 0:64] = np.eye(64, dtype=f32)
    pk32[0:64, 64:128] = g["proj_w"]
    pk32[:, 128] = kvb
    pk32[:, 129:131] = b1h
    pk32[0:64, 131] = bqv
    pk32[0:64, 132] = g["proj_b"]
    pk32[0:64, 133] = g["b2"]
    pk32[0:64, 134:262] = wqkv_g[:, 64:192]
    sel = np.zeros((32, 2048), np.float16)
    for b in range(B):
        for m in range(4):
            c0 = (b * 4 + m) * 128
            sel[b * 8 + 2 * m, c0:c0 + 64] = 1.0
            sel[b * 8 + 2 * m + 1, c0 + 64:c0 + 128] = 1.0
    pk16 = np.zeros((128, 3200), np.float16)
    pk16[0:64, 3136:3200] = np.eye(64, dtype=np.float16)
    pk16[0:32, 0:2048] = sel
    pk16[:, 2048:2112] = g["w2"][0:128, :].astype(np.float16)
    pk16[:, 2112:2176] = g["w2"][128:256, :].astype(np.float16)
    pk16[0:64, 2176:2368] = wqkv_g.astype(np.float16)
    pk16[0:64, 2368:2624] = w1_g.astype(np.float16)
    for b in range(B):
        for m in range(4):
            c0 = 2624 + (b * 4 + m) * 32
            pk16[0:64, c0 + b * 8 + 2 * m] = 1.0
            pk16[64:128, c0 + b * 8 + 2 * m + 1] = 1.0
    shared = {"pk32": pk32, "pk16": np.ascontiguousarray(pk16)}
    in_maps = []
    for k in range(N_CORES):
        i0 = k * IPC
        m = dict(shared)
        # x: rows (b,i,j) -> [p=(i%2,j), (chunk, b, q=i//2, t_chunk)]
        xv = x[:, i0:i0 + IPC].reshape(B, 4, 128, T).astype(np.float16)
        m["x"] = np.concatenate(
            [xv[:, :, :, c0:c0 + cw].transpose(2, 0, 1, 3).reshape(128, -1)
             for c0, cw in CHH], axis=1)
        # fw: [p, (chunk, m, t_chunk)]
        fv = fw[i0:i0 + IPC].reshape(4, 128, T).astype(np.float16)
        m["fw"] = np.concatenate(
            [fv[:, :, c0:c0 + cw].transpose(1, 0, 2).reshape(128, -1)
             for c0, cw in CHH], axis=1)
        # booster output is sharded over j (rev[b,i,j]=bw[i,j]*hr[b,j]):
        # rows (j_loc, i) so the broadcast h row per 64-row group is local
        m["bw"] = np.ascontiguousarray(
            bw[:, i0:i0 + IPC].transpose(1, 0, 2).reshape(
                4, 128, T).transpose(1, 0, 2).reshape(
                128, 4 * T)).astype(np.float16)
        in_maps.append(m)
    return in_maps


def _get_nc():
    if "nc" not in _CACHE:
        _CACHE["nc"] = _build_nc()
    return _CACHE["nc"]


def run(inputs, trace=False, **kw):
    nc = _get_nc()
    in_maps = _prep_host(inputs)
    res = run_bass_kernel_spmd(nc, in_maps, core_ids=list(range(N_CORES)),
                               trace=trace, **kw)
    full = np.empty((B, SD, SD, T), dtype=np.float32)
    for k in range(N_CORES):
        i0 = k * IPC
        co = res.results[k]["out"].astype(np.float32).reshape(B, IPC, SD, T)
        full[:, :, i0:i0 + IPC] = co.transpose(0, 2, 1, 3)
    return full.reshape(B, T, NE), res


def kernel(**inputs):
    out, _ = run(inputs)
    return out
